# revision 36
# baseline (speedup 1.0000x reference)
"""Fused quantized BasicBlock (1-bit weights / 4-bit acts) for TRN2, 8-core data-parallel.

Math: both convs see integer activations k in {0..15} (exactly representable in
fp8e4) and sign weights in {-1,0,+1}; the 3x3 conv is 9 shifted DoubleRow fp8
matmuls (K=256 contraction in one pass) accumulating exactly in fp32 PSUM.
All scalings (LSQ alpha, IR-Net weight scale, BN affine) fold into a
per-output-channel affine applied in the epilogue.

Layout: activations live in SBUF as fp8 integers in a zero-padded [60 x 58]
image (data at rows/cols 1..56, stride 58). A 3x3 tap (kh,kw) is then just a
contiguous 464-byte-per-k-chunk slice at offset (r0+kh)*58+kw covering 8 output
rows; the 2 pad columns per row produce garbage PSUM columns that the epilogue
never reads.
"""

import numpy as np
import ml_dtypes

import concourse.bass as bass
import concourse.bacc as bacc
import concourse.mybir as mybir
from concourse.tile import TileContext
from concourse.tile_rust import add_dep_helper
from concourse.bass_utils import run_bass_kernel_spmd

F32 = mybir.dt.float32
FP8 = mybir.dt.float8e4
NP_FP8 = ml_dtypes.float8_e4m3
AF = mybir.ActivationFunctionType
ALU = mybir.AluOpType
DR = mybir.MatmulPerfMode.DoubleRow

B, C, H, W = 32, 256, 56, 56
N_CORES = 8
BPC = B // N_CORES          # images per core
PW, PH = 58, 60             # padded image: 60 rows x 58 cols, data at [1:57, 1:57]
KCH = 3520                  # bytes per k-chunk (>= PH*PW, multiple of 16 for DoubleRow)
KCH_A = 1568                # image-0 top band: padded rows 0..25 + wrap row 26
KCH_B = 2032                # image-0 bottom band: padded rows 24..57 + wrap row (local 0..34)
NMM = 8 * PW                # moving free dim per matmul: 8 output rows
MAGIC = float(np.float32(2.0 ** 23))  # fp32 add/sub of 2^23 == round-to-nearest-even
QMAX = 15.0

_module_cache = {}


def _emit_memset_pads(nc, kt):
    """Zero the padding borders of one [128, 2, KCH] activation tile."""
    for cc in (0, 1):
        v = kt[:, cc, :]
        nc.vector.memset(v[:, 0:PW], 0.0)                    # row 0
        nc.vector.memset(v[:, 57 * PW:PH * PW], 0.0)         # rows 57..59
        vv = v[:, 0:PH * PW].rearrange("p (r c) -> p r c", c=PW)
        nc.vector.memset(vv[:, 1:57, 0:1], 0.0)              # col 0
        nc.vector.memset(vv[:, 1:57, 57:58], 0.0)            # col 57


def _dep_all(insts, prereqs, reason):
    """Each inst in `insts` waits for every DMA in `prereqs` to finish, so a
    later DMA wave can't round-robin-steal queue bandwidth from a critical
    earlier one."""
    for a in insts:
        for b in prereqs:
            add_dep_helper(a.ins, b.ins, reason=reason)


def _emit_x_dma(nc, xr, i, xpool, nrows, tag, bufs, after=()):
    """Issue the DMAs for image i's input, in row-chunks of nrows.
    bufs must cover all chunks in flight so triggers don't stall on slots."""
    tiles = {}
    dmas = []
    for cc in (0, 1):
        for rr0 in range(0, H, nrows):
            xt = xpool.tile([128, nrows * W], F32, tag=tag, bufs=bufs,
                            name=f"x_{i}_{cc}_{rr0}")
            dma = nc.sync.dma_start(
                out=xt[:], in_=xr[i, cc][:, rr0 * W:(rr0 + nrows) * W])
            dmas.append(dma)
            tiles[(cc, rr0)] = xt
    _dep_all(dmas, after, f"x{i} waits prior DMA wave")
    return tiles, dmas


def _emit_quant_input(nc, i, xtiles, tqp, rqp, k1t, coef_t, nrows):
    """k1 = min(rne(relu(x/alpha1)), 15) as fp8 into padded layout."""
    for cc in (0, 1):
        for rr0 in range(0, H, nrows):
            xt = xtiles[(cc, rr0)]
            tq = tqp.tile([128, nrows * W], F32, tag="tq")
            nc.scalar.activation(out=tq[:], in_=xt[:],
                                 func=AF.Relu, scale=coef_t[:, 8:9])
            rq = rqp.tile([128, nrows * W], F32, tag="rq")
            nc.vector.tensor_scalar(
                out=rq[:], in0=tq[:], scalar1=MAGIC, scalar2=MAGIC,
                op0=ALU.add, op1=ALU.subtract)
            dst = k1t[i][:, cc, 0:PH * PW].rearrange("p (r c) -> p r c", c=PW)[
                :, rr0 + 1:rr0 + 1 + nrows, 1:57]
            nc.vector.tensor_scalar_min(
                dst, rq[:].rearrange("p (r c) -> p r c", c=W), QMAX)


def _emit_conv(nc, i, wt, kin, psum, layer2, ep1p, ep2p, stp, k2t, o_r, coef_t,
               kin_banded=None):
    """One 3x3 conv layer for image i: 9 shifted DoubleRow matmuls per output tile.

    kin_banded: optional rb -> (tile, base_padded_row) override so early row
    blocks can depend on a partially-quantized input (startup pipelining)."""
    if layer2 and i == BPC - 1:
        # split the very last row block so the final post-matmul epilogue+DMA
        # (serial tail after the last MM) is as small as possible
        blocks = [(r0, 8) for r0 in range(0, 48, 8)] + [(48, 6), (54, 2)]
    else:
        blocks = [(r0, 8) for r0 in range(0, 56, 8)]
    for r0, nr in blocks:
        for occ in (0, 1):
            if kin_banded is not None:
                ktile, base = kin_banded(r0 // 8)
            else:
                ktile, base = kin[i], 0
            nmm = nr * PW
            ps = psum.tile([128, NMM], F32, tag="ps")
            for off in range(9):
                kh, kw = divmod(off, 3)
                s = (r0 + kh - base) * PW + kw
                nc.tensor.matmul(
                    ps[:, 0:nmm], wt[occ][:, :, off, :], ktile[:, :, s:s + nmm],
                    start=(off == 0), stop=(off == 8), perf_mode=DR)
            psv = ps[:, 0:nmm].rearrange("p (r c) -> p r c", c=PW)[:, :, 0:56]
            if not layer2:
                # k2 = min(rne(relu((A1/a2)*conv + B1/a2)), 15) -> fp8, all on DVE
                # (ACT is the scarcer engine: it owns input quant + final epilogue)
                t1 = ep1p.tile([128, 8 * 56], F32, tag="ep1")
                nc.vector.tensor_scalar(
                    out=t1[:, 0:nr * 56].rearrange("p (r c) -> p r c", c=56), in0=psv,
                    scalar1=coef_t[:, occ:occ + 1], scalar2=coef_t[:, 2 + occ:3 + occ],
                    op0=ALU.mult, op1=ALU.add)
                t2 = ep2p.tile([128, 8 * 56], F32, tag="ep2")
                nc.vector.tensor_scalar(
                    out=t2[:, 0:nr * 56], in0=t1[:, 0:nr * 56], scalar1=0.0,
                    scalar2=MAGIC, op0=ALU.max, op1=ALU.add)
                dst = k2t[i][:, occ, 0:PH * PW].rearrange("p (r c) -> p r c", c=PW)[
                    :, r0 + 1:r0 + 1 + nr, 1:57]
                nc.vector.tensor_scalar(
                    out=dst,
                    in0=t2[:, 0:nr * 56].rearrange("p (r c) -> p r c", c=56),
                    scalar1=MAGIC, scalar2=QMAX,
                    op0=ALU.subtract, op1=ALU.min)
            else:
                # out = relu(A2*conv + B2) on ACT, then DMA to DRAM
                st = stp.tile([128, 8 * 56], F32, tag="st")
                nc.scalar.activation(
                    out=st[:, 0:nr * 56].rearrange("p (r c) -> p r c", c=56), in_=psv,
                    func=AF.Relu, scale=coef_t[:, 4 + occ:5 + occ],
                    bias=coef_t[:, 6 + occ:7 + occ])
                nc.sync.dma_start(
                    out=o_r[i, occ][:, r0 * 56:(r0 + nr) * 56], in_=st[:, 0:nr * 56])


def _build_module():
    # Bacc (not raw Bass): its compile() legalizes multi-sem waits (TRN2 allows
    # one wait per instruction) and inserts activation table loads.
    nc = bacc.Bacc("TRN2", debug=False)
    x_d = nc.dram_tensor("x", [BPC, C, H, W], F32, kind="ExternalInput")
    w1_d = nc.dram_tensor("w1p", [2, 128, 2, 9, 128], FP8, kind="ExternalInput")
    w2_d = nc.dram_tensor("w2p", [2, 128, 2, 9, 128], FP8, kind="ExternalInput")
    cf_d = nc.dram_tensor("coef", [128, 9], F32, kind="ExternalInput")
    o_d = nc.dram_tensor("out", [BPC, C, H, W], F32, kind="ExternalOutput")

    xr = x_d.ap().rearrange("b (cc p) h w -> b cc p (h w)", p=128)
    o_r = o_d.ap().rearrange("b (cc p) h w -> b cc p (h w)", p=128)

    with TileContext(nc) as tc:
        with tc.tile_pool(name="weights", bufs=1) as wpool, \
             tc.tile_pool(name="acts", bufs=1) as kpool, \
             tc.tile_pool(name="xin", bufs=4) as xpool, \
             tc.tile_pool(name="tq", bufs=2) as tqp, \
             tc.tile_pool(name="rq", bufs=2) as rqp, \
             tc.tile_pool(name="ep1", bufs=4) as ep1p, \
             tc.tile_pool(name="ep2", bufs=4) as ep2p, \
             tc.tile_pool(name="st", bufs=4) as stp, \
             tc.tile_pool(name="coef", bufs=1) as cfp, \
             tc.tile_pool(name="psum", bufs=8, space="PSUM") as psum:

            # coef first (tiny), then image 0's top-band input chunks: nothing
            # else ahead of them in the DMA queues — the first matmul gates
            # on image 0's top band.
            coef_t = cfp.tile([128, 9], F32, tag="coef")
            nc.sync.dma_start(out=coef_t[:], in_=cf_d.ap())

            # image 0's critical top-band chunks: cc0 on the ACT HWDGE queue,
            # cc1 on the Sync HWDGE queue — two parallel DMA paths with
            # nothing else competing for them.
            # image 0's input in 3 row-chunks per k-chunk: 18/24/14 pixel rows.
            # Only the first (1.03MB total) gates the first matmul.
            X0R = {0: (0, 18), 1: (18, 24), 2: (42, 14)}
            x0 = {}
            x0_dmas = {0: [], 1: [], 2: []}
            for ci in (0, 1, 2):
                p0, nr = X0R[ci]
                for cc in (0, 1):
                    xt = xpool.tile([128, nr * W], F32, tag="xin0", bufs=6,
                                    name=f"x_0_{cc}_{ci}")
                    x0_dmas[ci].append(nc.sync.dma_start(
                        out=xt[:], in_=xr[0, cc][:, p0 * W:(p0 + nr) * W]))
                    x0[(cc, ci)] = xt
                if ci == 1:
                    _dep_all(x0_dmas[1], x0_dmas[0], "x0 c1 after c0")
                if ci == 2:
                    _dep_all(x0_dmas[2], x0_dmas[1], "x0 c2 after c1")

            # dummy activation gated only on the coef DMA: pulls the one-time
            # ACT_TABLE_LOAD (~1.3us) off the quant critical path.
            scr = cfp.tile([128, 1], F32, tag="scr")
            nc.scalar.activation(out=scr[:], in_=coef_t[:, 0:1], func=AF.Relu)

            # w1 on the otherwise-empty ACT HWDGE queue: arrives ~10us without
            # competing with the critical x chunks on the Sync queue
            w1t, w2t = [], []
            for occ in (0, 1):
                t = wpool.tile([128, 2, 9, 128], FP8, tag=f"w1_{occ}", name=f"w1_{occ}")
                nc.scalar.dma_start(out=t[:], in_=w1_d.ap()[occ])
                w1t.append(t)

            # image 0's layer-1 input lives in 4 band tiles so row-block
            # matmuls start as soon as their rows are quantized:
            #   A1: rb0-1 <- padded rows 0..18  (19 rows, data 1..18)
            #   A2: rb2   <- padded rows 16..26 (11 rows, base 16)
            #   B1: rb3-4 <- padded rows 24..42 (19 rows, base 24)
            #   B2: rb5-6 <- padded rows 40..58 (19 rows, base 40)
            KB = 1104
            bands = {}
            for name in ("A1", "A2", "B1", "B2"):
                sz = 640 if name == "A2" else KB
                bands[name] = kpool.tile([128, 2, sz], FP8, tag=f"kb_{name}",
                                         name=f"kb_{name}")
            k1t, k2t = [None], []
            for i in range(BPC):
                if i > 0:
                    k1t.append(kpool.tile([128, 2, KCH], FP8, tag=f"k1_{i}",
                                          name=f"k1_{i}"))
                k2t.append(kpool.tile([128, 2, KCH], FP8, tag=f"k2_{i}", name=f"k2_{i}"))

            def bview(name, cc, lo, hi):
                nrows = 11 if name == "A2" else 19
                return bands[name][:, cc, 0:nrows * PW].rearrange(
                    "p (r c) -> p r c", c=PW)[:, lo:hi, 1:57]

            # band pad memsets (data rows per band; zero rows + col pads)
            for cc in (0, 1):
                a1 = bands["A1"][:, cc, :]
                nc.vector.memset(a1[:, 0:PW], 0.0)                  # padded row 0
                va1 = a1[:, 0:19 * PW].rearrange("p (r c) -> p r c", c=PW)
                nc.vector.memset(va1[:, 1:19, 0:1], 0.0)
                nc.vector.memset(va1[:, 1:19, 57:58], 0.0)
                a2 = bands["A2"][:, cc, :]
                va2 = a2[:, 0:11 * PW].rearrange("p (r c) -> p r c", c=PW)
                nc.vector.memset(va2[:, 0:11, 0:1], 0.0)
                nc.vector.memset(va2[:, 0:11, 57:58], 0.0)
                b1 = bands["B1"][:, cc, :]
                vb1 = b1[:, 0:19 * PW].rearrange("p (r c) -> p r c", c=PW)
                nc.vector.memset(vb1[:, 0:19, 0:1], 0.0)
                nc.vector.memset(vb1[:, 0:19, 57:58], 0.0)
                b2 = bands["B2"][:, cc, :]
                nc.vector.memset(b2[:, 17 * PW:KB], 0.0)            # rows 57, 58
                vb2 = b2[:, 0:19 * PW].rearrange("p (r c) -> p r c", c=PW)
                nc.vector.memset(vb2[:, 0:17, 0:1], 0.0)
                nc.vector.memset(vb2[:, 0:17, 57:58], 0.0)

            # quantize image 0: chunk ci covers pixel rows [p0, p0+nr);
            # dst list: (band, local_lo, local_hi, src_lo, src_hi)
            chunk_dsts = {
                0: [("A1", 1, 19, 0, 18), ("A2", 0, 3, 15, 18)],
                1: [("A2", 3, 11, 0, 8), ("B1", 0, 19, 5, 24), ("B2", 0, 3, 21, 24)],
                2: [("B2", 3, 17, 0, 14)],
            }
            for ci in (0, 1, 2):
                p0, nr = X0R[ci]
                for cc in (0, 1):
                    tq = tqp.tile([128, nr * W], F32, tag="tq")
                    nc.scalar.activation(out=tq[:], in_=x0[(cc, ci)][:],
                                         func=AF.Relu, scale=coef_t[:, 8:9])
                    rq = rqp.tile([128, nr * W], F32, tag="rq")
                    nc.vector.tensor_scalar(
                        out=rq[:], in0=tq[:], scalar1=MAGIC, scalar2=MAGIC,
                        op0=ALU.add, op1=ALU.subtract)
                    rqv = rq[:].rearrange("p (r c) -> p r c", c=W)
                    for bname, lo, hi, slo, shi in chunk_dsts[ci]:
                        nc.vector.tensor_scalar_min(
                            bview(bname, cc, lo, hi), rqv[:, slo:shi], QMAX)
            _emit_memset_pads(nc, k2t[0])

            def quant(i, xtiles, nrows):
                _emit_memset_pads(nc, k1t[i])
                _emit_quant_input(nc, i, xtiles, tqp, rqp, k1t, coef_t, nrows)
                _emit_memset_pads(nc, k2t[i])

            _band_of_rb = {0: ("A1", 0), 1: ("A1", 0), 2: ("A2", 16),
                           3: ("B1", 24), 4: ("B1", 24),
                           5: ("B2", 40), 6: ("B2", 40)}

            def l1(i):
                banded = (lambda rb: (bands[_band_of_rb[rb][0]],
                                      _band_of_rb[rb][1])) if i == 0 else None
                _emit_conv(nc, i, w1t, k1t, psum, False, ep1p, ep2p, stp,
                           k2t, o_r, coef_t, kin_banded=banded)

            def l2(i):
                _emit_conv(nc, i, w2t, k2t, psum, True, ep1p, ep2p, stp,
                           None, o_r, coef_t)

            # stagger so PE never waits: image i's L1 can start while image
            # i+1 still quantizes; L2(i) runs after L1(i)'s epilogues.
            # DMA waves chained with explicit deps so each wave gets full
            # queue bandwidth.
            x1, x1_dmas = _emit_x_dma(nc, xr, 1, xpool, 28, 'xin', 4,
                                      after=x0_dmas[2])
            w2_dmas = []
            for occ in (0, 1):
                t = wpool.tile([128, 2, 9, 128], FP8, tag=f"w2_{occ}", name=f"w2_{occ}")
                w2_dmas.append(nc.sync.dma_start(out=t[:], in_=w2_d.ap()[occ]))
                w2t.append(t)
            _dep_all(w2_dmas, x1_dmas, "w2 after x1")
            quant(1, x1, 28)
            l1(0)
            x2, x2_dmas = _emit_x_dma(nc, xr, 2, xpool, 28, 'xin', 4,
                                      after=x1_dmas)
            quant(2, x2, 28)
            l1(1); l2(0)
            x3, x3_dmas = _emit_x_dma(nc, xr, 3, xpool, 28, 'xin', 4,
                                      after=x2_dmas)
            quant(3, x3, 28)
            l1(2); l2(1)
            l1(3); l2(2)
            l2(3)

    nc.compile()
    return nc


def get_module():
    if "nc" not in _module_cache:
        _module_cache["nc"] = _build_module()
    return _module_cache["nc"]


def _binarize(w):
    """IR-Net forward: sign(normalized w) and per-out-channel scale (fp32)."""
    w = np.asarray(w, np.float32)
    mu = w.mean(axis=(1, 2, 3), keepdims=True, dtype=np.float32)
    var = ((w - mu) ** 2).mean(axis=(1, 2, 3), keepdims=True, dtype=np.float32)
    std = np.sqrt(var)
    wn = (w - mu) / (std + np.float32(1e-5))
    sgn = np.sign(wn).astype(np.float32)
    scale = np.abs(wn).mean(axis=(1, 2, 3), dtype=np.float32)  # [O]
    return sgn, scale


def _pack_weights(sgn):
    """[O=256, C=256, 3, 3] signs -> [occ, p(Ki), h(Ko), off, m] fp8 with c = h*128+p."""
    s = sgn.reshape(256, 256, 9)
    s = s.reshape(2, 128, 2, 128, 9)            # [occ, m, h, p, off]
    s = np.transpose(s, (0, 3, 2, 4, 1))        # [occ, p, h, off, m]
    return np.ascontiguousarray(s).astype(NP_FP8)


def kernel(x, w1, alpha1, g1, b1, m1, v1, w2, alpha2, g2, b2, m2, v2,
           _trace=False):
    f32 = np.float32
    x = np.asarray(x, f32)
    a1 = f32(np.asarray(alpha1).reshape(()))
    a2 = f32(np.asarray(alpha2).reshape(()))
    g1, b1, m1, v1 = (np.asarray(t, f32) for t in (g1, b1, m1, v1))
    g2, b2, m2, v2 = (np.asarray(t, f32) for t in (g2, b2, m2, v2))

    s1, sc1 = _binarize(w1)
    s2, sc2 = _binarize(w2)
    inv1 = g1 / np.sqrt(v1 + f32(1e-5))
    inv2 = g2 / np.sqrt(v2 + f32(1e-5))

    A1 = (a1 * sc1 * inv1 / a2).astype(f32)         # folds layer2 1/alpha in
    B1 = ((b1 - m1 * inv1) / a2).astype(f32)
    A2 = (a2 * sc2 * inv2).astype(f32)
    B2 = (b2 - m2 * inv2).astype(f32)

    coef = np.zeros((9, 128), f32)
    coef[0:2] = A1.reshape(2, 128)
    coef[2:4] = B1.reshape(2, 128)
    coef[4:6] = A2.reshape(2, 128)
    coef[6:8] = B2.reshape(2, 128)
    coef[8] = f32(1.0) / a1

    coef = np.ascontiguousarray(coef.T)   # [128, 9]: contiguous per-partition DMA

    w1p = _pack_weights(s1)
    w2p = _pack_weights(s2)

    nc = get_module()
    in_maps = [
        {"x": np.ascontiguousarray(x[i * BPC:(i + 1) * BPC]),
         "w1p": w1p, "w2p": w2p, "coef": coef}
        for i in range(N_CORES)
    ]
    res = run_bass_kernel_spmd(nc, in_maps, core_ids=list(range(N_CORES)),
                               trace=_trace)
    out = np.concatenate([r["out"] for r in res.results], axis=0)
    if _trace:
        return out, res
    return out
